# revision 18
# baseline (speedup 1.0000x reference)
"""CLAM-SB attention-MIL kernel for 8 Trainium2 NeuronCores.

Model (reference):
    feat  = relu(h @ W1 + b1)                      # [N, 512]
    a     = tanh(feat @ Wa + ba)                   # [N, 256]
    g     = sigmoid(feat @ Wb + bb)                # [N, 256]
    A     = (a*g) @ Wc + bc                        # [N, 1]
    sig   = sigmoid(A);  atten = sig / sum(sig)
    slide_logits = (atten @ feat) @ Wcls + bcls    # [1, 2]
    inst_loss    = CE over top-8 / bottom-8 rows of feat @ Winst + binst

Sharding: instance dim N=50000 split 6250/core over 8 cores, padded to
6656 = 13*512.  Each core receives its h-shard TRANSPOSED (hT [1024, 6656])
so every on-device matmul has its contraction dim on partitions with no
on-device transposes at all.  All tensors live feature-major ("T" layout):
featT [512, i], aT/gT [256, i], A row [1, i].  In this layout every bias is
a per-partition scalar and rides the activation instruction for free.

Per core the device returns: A scores [1, 6656], per-block top-8/bottom-8
candidates (Max8 values + indices, computed inline so there is no serial
top-k tail), the partial attention-weighted feature sum z [512] and the
partial sum-of-sigmoids.  The host merges the per-shard candidates, re-ranks
them with an exact recompute of just those rows (the device's
reduced-precision A cannot be trusted to order near-ties at the k-th
boundary), and finishes the tiny CE / softmax epilogue.
"""

import numpy as np
import ml_dtypes

import concourse.bass as bass
import concourse.mybir as mybir
import concourse.tile as tile
from concourse import bacc
from concourse.bass_utils import run_bass_kernel_spmd

N_CORES = 8
N_TOTAL = 50000
ROWS = N_TOTAL // N_CORES          # 6250 per core
NB = 13                            # i-blocks per core
BLK = 512
ROWS_PAD = NB * BLK                # 6656
L = 1024                           # input feature dim
D1 = 512                           # feature dim
D2 = 256                           # attention dim
K_SAMPLE = 8
F32 = mybir.dt.float32
BF16 = mybir.dt.bfloat16
U32 = mybir.dt.uint32
PAD_START_IN_LAST = ROWS - (NB - 1) * BLK   # = 106

# Matmul dtype; float32r runs the PE at full rate (vs 1/4 for float32) at
# ~tf32 precision.  The BIR verifier requires every tensor consumed by an
# fp32r matmul to be produced as float32r, so matmul-feeding tiles are
# declared MM_DT (DMA and the scalar engine may produce it; the vector
# engine may not, hence the A-scores matmul runs on bf16 operands).
MM_DT = mybir.dt.float32r

LAST_RESULT = None  # BassKernelResults of the most recent kernel() call


def build_nc(bc_val: float):
    nc = bacc.Bacc("TRN2", target_bir_lowering=False, debug=False,
                   num_devices=N_CORES)

    hT = nc.dram_tensor("hT", [L, ROWS_PAD], BF16, kind="ExternalInput").ap()
    W1 = nc.dram_tensor("W1", [128, 8 * D1], BF16, kind="ExternalInput").ap()
    Wa = nc.dram_tensor("Wa", [128, 4 * D2], BF16, kind="ExternalInput").ap()
    Wb = nc.dram_tensor("Wb", [128, 4 * D2], BF16, kind="ExternalInput").ap()
    Wc = nc.dram_tensor("Wc", [128, 2], BF16, kind="ExternalInput").ap()
    ones = nc.dram_tensor("ones", [1, 128], MM_DT, kind="ExternalInput").ap()
    b1 = nc.dram_tensor("b1", [128, 4], F32, kind="ExternalInput").ap()
    ba = nc.dram_tensor("ba", [128, 2], F32, kind="ExternalInput").ap()
    bb = nc.dram_tensor("bb", [128, 2], F32, kind="ExternalInput").ap()

    A_out = nc.dram_tensor("A_out", [1, ROWS_PAD], F32, kind="ExternalOutput").ap()
    cvt_out = nc.dram_tensor("cvt_out", [1, NB * 8], F32, kind="ExternalOutput").ap()
    cit_out = nc.dram_tensor("cit_out", [1, NB * 8], U32, kind="ExternalOutput").ap()
    cvb_out = nc.dram_tensor("cvb_out", [1, NB * 8], F32, kind="ExternalOutput").ap()
    cib_out = nc.dram_tensor("cib_out", [1, NB * 8], U32, kind="ExternalOutput").ap()
    z_out = nc.dram_tensor("z_out", [128, 4], F32, kind="ExternalOutput").ap()
    ssum_out = nc.dram_tensor("ssum_out", [1, 1], F32, kind="ExternalOutput").ap()

    with tile.TileContext(nc) as tc:
        with (
            tc.tile_pool(name="const", bufs=1) as cpool,
            tc.tile_pool(name="persist", bufs=1) as ppool,
            tc.tile_pool(name="xt", bufs=3) as xtpool,
            tc.tile_pool(name="work", bufs=2) as wpool,
            tc.tile_pool(name="feat_ps", bufs=3, space="PSUM") as fpspool,
            tc.tile_pool(name="ag_ps", bufs=3, space="PSUM") as agpspool,
            tc.tile_pool(name="small_ps", bufs=1, space="PSUM") as spspool,
        ):
            # ---- resident constants -------------------------------------
            W1_sb = cpool.tile([128, 8, D1], BF16)       # [k_in_j, j, d1]
            nc.scalar.dma_start(W1_sb[:], W1.rearrange("p (j d) -> p j d", j=8))
            Wa_sb = cpool.tile([128, 4, D2], BF16)       # [d1_in_m, m, d2]
            nc.scalar.dma_start(Wa_sb[:], Wa.rearrange("p (m s) -> p m s", m=4))
            Wb_sb = cpool.tile([128, 4, D2], BF16)
            nc.scalar.dma_start(Wb_sb[:], Wb.rearrange("p (m s) -> p m s", m=4))
            Wc_sb = cpool.tile([128, 2], BF16)            # [d2_in_s, s]
            nc.scalar.dma_start(Wc_sb[:], Wc)
            b1_sb = cpool.tile([128, 4], F32)
            nc.scalar.dma_start(b1_sb[:], b1)
            ba_sb = cpool.tile([128, 2], F32)
            nc.scalar.dma_start(ba_sb[:], ba)
            bb_sb = cpool.tile([128, 2], F32)
            nc.scalar.dma_start(bb_sb[:], bb)
            ones_row = cpool.tile([1, 128], MM_DT)
            nc.scalar.dma_start(ones_row[:], ones)

            # ---- persistent accumulators --------------------------------
            A_all = ppool.tile([1, ROWS_PAD], F32)
            ssum_parts = ppool.tile([1, NB], F32)
            z_parts = ppool.tile([128, 4, NB], F32)
            cvt = ppool.tile([1, NB * 8], F32)
            cit = ppool.tile([1, NB * 8], U32)
            cvb = ppool.tile([1, NB * 8], F32)
            cib = ppool.tile([1, NB * 8], U32)

            hT_r = hT.rearrange("(j p) n -> p j n", p=128)

            for b in range(NB):
                last = b == NB - 1
                i0 = b * BLK
                isl = slice(i0, i0 + BLK)
                ksl = slice(b * 8, (b + 1) * 8)

                xt = xtpool.tile([128, 8, BLK], BF16, tag="xt")
                nc.sync.dma_start(xt[:], hT_r[:, :, isl])

                # featT[m] = relu(sum_j W1[j,m].T @ Xt[j] + b1[m])
                featT = wpool.tile([128, 4, BLK], BF16, tag="featT")
                for m in range(4):
                    fps = fpspool.tile([128, BLK], F32, tag="fps")
                    for j in range(8):
                        nc.tensor.matmul(
                            fps[:], W1_sb[:, j, m * 128:(m + 1) * 128],
                            xt[:, j, :], start=(j == 0), stop=(j == 7))
                    nc.scalar.activation(
                        featT[:, m, :], fps[:],
                        mybir.ActivationFunctionType.Relu,
                        bias=b1_sb[:, m:m + 1])

                # aT[s] = tanh(sum_m Wa[m,s].T @ featT[m] + ba[s]); gT likewise
                aT = wpool.tile([128, 2, BLK], F32, tag="aT")
                gT = wpool.tile([128, 2, BLK], F32, tag="gT")
                for dst, Wsb, bsb, fn in (
                    (aT, Wa_sb, ba_sb, mybir.ActivationFunctionType.Tanh),
                    (gT, Wb_sb, bb_sb, mybir.ActivationFunctionType.Sigmoid),
                ):
                    for s in range(2):
                        ps = agpspool.tile([128, BLK], F32, tag="agps")
                        for m in range(4):
                            nc.tensor.matmul(
                                ps[:], Wsb[:, m, s * 128:(s + 1) * 128],
                                featT[:, m, :], start=(m == 0), stop=(m == 3))
                        nc.scalar.activation(dst[:, s, :], ps[:], fn,
                                             bias=bsb[:, s:s + 1])

                agT = wpool.tile([128, 2, BLK], BF16, tag="agT")
                nc.vector.tensor_mul(agT[:], aT[:], gT[:])

                # A row = sum_s Wc[s].T @ agT[s]  (+ bc via activation bias)
                aps = spspool.tile([1, BLK], F32, tag="aps")
                for s in range(2):
                    nc.tensor.matmul(aps[:], Wc_sb[:, s:s + 1],
                                     agT[:, s, :], start=(s == 0), stop=(s == 1))
                nc.scalar.activation(A_all[0:1, isl], aps[:],
                                     mybir.ActivationFunctionType.Identity,
                                     bias=float(bc_val))

                # per-block top/bottom-8 candidates (values + in-block index);
                # the host adds the block offset and filters pad indices
                An = wpool.tile([1, BLK], F32, tag="An")
                nc.vector.tensor_scalar_mul(An[:], A_all[0:1, isl], -1.0)
                nc.vector.max(cvt[0:1, ksl], A_all[0:1, isl])
                nc.vector.max_index(cit[0:1, ksl], cvt[0:1, ksl], A_all[0:1, isl])
                nc.vector.max(cvb[0:1, ksl], An[:])
                nc.vector.max_index(cib[0:1, ksl], cvb[0:1, ksl], An[:])

                # sig row (+ partial sum); padded tail masked to 0
                sig = wpool.tile([1, BLK], MM_DT, tag="sig")
                if not last:
                    nc.scalar.activation(sig[:], aps[:],
                                         mybir.ActivationFunctionType.Sigmoid,
                                         bias=float(bc_val),
                                         accum_out=ssum_parts[0:1, b:b + 1])
                else:
                    nc.scalar.activation(sig[:], aps[:],
                                         mybir.ActivationFunctionType.Sigmoid,
                                         bias=float(bc_val))
                    nc.vector.memset(sig[0:1, PAD_START_IN_LAST:BLK].bitcast(F32), 0.0)
                    nc.vector.reduce_sum(ssum_parts[0:1, b:b + 1], sig[:].bitcast(F32),
                                         axis=mybir.AxisListType.X)

                # z partials: broadcast sig across partitions via PE, then
                # per-d1-slice multiply-reduce on DVE
                bps = spspool.tile([128, BLK], F32, tag="bps")
                nc.tensor.matmul(bps[:], ones_row[:], sig[:],
                                 start=True, stop=True)
                zscr = wpool.tile([128, BLK], F32, tag="zscr")
                for m in range(4):
                    nc.vector.scalar_tensor_tensor(
                        out=zscr[:], in0=featT[:, m, :], scalar=1.0,
                        in1=bps[:],
                        op0=mybir.AluOpType.mult, op1=mybir.AluOpType.mult,
                        accum_out=z_parts[:, m, b:b + 1])

            # ---- tails ---------------------------------------------------
            nc.gpsimd.dma_start(A_out, A_all[:])
            nc.gpsimd.dma_start(cvt_out, cvt[:])
            nc.gpsimd.dma_start(cit_out, cit[:])
            nc.gpsimd.dma_start(cvb_out, cvb[:])
            nc.gpsimd.dma_start(cib_out, cib[:])

            z_sb = ppool.tile([128, 4], F32)
            for m in range(4):
                nc.vector.reduce_sum(z_sb[:, m:m + 1], z_parts[:, m, :],
                                     axis=mybir.AxisListType.X)
            nc.gpsimd.dma_start(z_out, z_sb[:])

            ssum_sb = ppool.tile([1, 1], F32)
            nc.vector.reduce_sum(ssum_sb[:], ssum_parts[:],
                                 axis=mybir.AxisListType.X)
            nc.gpsimd.dma_start(ssum_out, ssum_sb[:])

    nc.compile()
    return nc


def kernel(h, W1, b1, Wa, ba, Wb, bb, Wc, bc, Wcls, bcls, Winst, binst, label):
    h = np.asarray(h, np.float32)
    W1 = np.asarray(W1, np.float32)
    Wa = np.asarray(Wa, np.float32)
    Wb = np.asarray(Wb, np.float32)
    Wc = np.asarray(Wc, np.float32)
    Wcls = np.asarray(Wcls, np.float32)
    Winst = np.asarray(Winst, np.float32)
    b1v = np.asarray(b1, np.float32).reshape(D1)
    bav = np.asarray(ba, np.float32).reshape(D2)
    bbv = np.asarray(bb, np.float32).reshape(D2)
    bc_val = float(np.asarray(bc, np.float32).reshape(-1)[0])

    nc = build_nc(bc_val)

    def klayout(w, parts):
        # [(t p), d] -> [p, t*d]: partition-major layout so the device DMA
        # is a flat contiguous [128, X] copy
        t = w.shape[0] // 128
        return np.ascontiguousarray(
            w.reshape(t, 128, -1).transpose(1, 0, 2).reshape(128, -1))

    shared = {
        "W1": klayout(W1, 8).astype(ml_dtypes.bfloat16),
        "Wa": klayout(Wa, 4).astype(ml_dtypes.bfloat16),
        "Wb": klayout(Wb, 4).astype(ml_dtypes.bfloat16),
        "Wc": klayout(Wc.reshape(D2, 1), 2).astype(ml_dtypes.bfloat16),
        "b1": klayout(b1v.reshape(D1, 1), 4).astype(np.float32),
        "ba": klayout(bav.reshape(D2, 1), 2).astype(np.float32),
        "bb": klayout(bbv.reshape(D2, 1), 2).astype(np.float32),
        "ones": np.ones((1, 128), np.float32),
    }
    in_maps = []
    for c in range(N_CORES):
        hT_c = np.zeros((L, ROWS_PAD), ml_dtypes.bfloat16)
        hT_c[:, :ROWS] = h[c * ROWS:(c + 1) * ROWS].T
        in_maps.append({"hT": np.ascontiguousarray(hT_c), **shared})

    res = run_bass_kernel_spmd(nc, in_maps, core_ids=list(range(N_CORES)))
    global LAST_RESULT
    LAST_RESULT = res

    # ---- host epilogue (gather / merge / tiny CE+softmax) ---------------
    z_full = np.zeros(D1, np.float64)
    ssum = 0.0
    cand = set()   # global indices nominated for the top side
    bcand = set()  # ... for the bottom side
    for c, r in enumerate(res.results):
        zc = r["z_out"]                      # [128, 4] -> z[m*128+p]
        z_full += zc.T.reshape(D1).astype(np.float64)
        ssum += float(r["ssum_out"][0, 0])
        ci_t = r["cit_out"][0].astype(np.int64)
        ci_b = r["cib_out"][0].astype(np.int64)
        for k in range(NB * 8):
            li = int(ci_t[k]) + (k // 8) * BLK
            if li < ROWS:
                cand.add(c * ROWS + li)
            li = int(ci_b[k]) + (k // 8) * BLK
            if li < ROWS:
                bcand.add(c * ROWS + li)

    # The device's reduced-precision A can flip near-tied candidates at the
    # k-th boundary, which visibly moves inst_loss.  Widen each side's
    # candidate set to the global top/bottom-64 by device A (the 8th<->64th
    # gap dwarfs the device A error), then re-rank the candidates with an
    # exact host recompute of their A scores and feature rows and run the
    # tiny CE on the exact values.
    A_dev = np.concatenate([r["A_out"][0, :ROWS] for r in res.results])
    WIDE = 64
    cand |= {int(g) for g in np.argpartition(-A_dev, WIDE)[:WIDE]}
    bcand |= {int(g) for g in np.argpartition(A_dev, WIDE)[:WIDE]}
    gidx = sorted(cand | bcand)
    hrows = h[gidx].astype(np.float64)
    feat_c = np.maximum(hrows @ W1 + b1v, 0)
    a_c = np.tanh(feat_c @ Wa + bav)
    g_c = 1.0 / (1.0 + np.exp(-(feat_c @ Wb + bbv)))
    A_c = ((a_c * g_c) @ Wc.reshape(D2, 1))[:, 0] + bc_val

    top_order = sorted((i for i, g in enumerate(gidx) if g in cand),
                       key=lambda i: (-A_c[i], gidx[i]))
    bot_order = sorted((i for i, g in enumerate(gidx) if g in bcand),
                       key=lambda i: (A_c[i], gidx[i]))
    sel = top_order[:K_SAMPLE] + bot_order[:K_SAMPLE]

    logits16 = feat_c[sel] @ Winst + np.asarray(binst, np.float64).reshape(1, 2)
    m = logits16.max(axis=1, keepdims=True)
    logp = logits16 - (m + np.log(np.exp(logits16 - m).sum(axis=1, keepdims=True)))
    targets = np.array([1] * K_SAMPLE + [0] * K_SAMPLE)
    inst_loss = -float(np.mean(logp[np.arange(2 * K_SAMPLE), targets]))
    inst_loss *= float(int(np.asarray(label).reshape(-1)[0]) == 1)

    zn = (z_full / ssum).astype(np.float32)          # [512] pooled feature
    slide_logits = (zn[None, :] @ Wcls + np.asarray(bcls, np.float32).reshape(1, 2))
    sl64 = slide_logits.astype(np.float64)
    e = np.exp(sl64 - sl64.max())
    slide_prob = (e / e.sum()).astype(np.float32)

    return (slide_logits.astype(np.float32),
            slide_prob.reshape(1, 2),
            np.float32(inst_loss))


# revision 19
# speedup vs baseline: 1.0101x; 1.0101x over previous
"""CLAM-SB attention-MIL kernel for 8 Trainium2 NeuronCores.

Model (reference):
    feat  = relu(h @ W1 + b1)                      # [N, 512]
    a     = tanh(feat @ Wa + ba)                   # [N, 256]
    g     = sigmoid(feat @ Wb + bb)                # [N, 256]
    A     = (a*g) @ Wc + bc                        # [N, 1]
    sig   = sigmoid(A);  atten = sig / sum(sig)
    slide_logits = (atten @ feat) @ Wcls + bcls    # [1, 2]
    inst_loss    = CE over top-8 / bottom-8 rows of feat @ Winst + binst

Sharding: instance dim N=50000 split 6250/core over 8 cores, padded to
6656 = 13*512.  Each core receives its h-shard TRANSPOSED (hT [1024, 6656])
so every on-device matmul has its contraction dim on partitions with no
on-device transposes at all.  All tensors live feature-major ("T" layout):
featT [512, i], aT/gT [256, i], A row [1, i].  In this layout every bias is
a per-partition scalar and rides the activation instruction for free.

Per core the device returns: A scores [1, 6656], per-block top-8/bottom-8
candidates (Max8 values + indices, computed inline so there is no serial
top-k tail), the partial attention-weighted feature sum z [512] and the
partial sum-of-sigmoids.  The host merges the per-shard candidates, re-ranks
them with an exact recompute of just those rows (the device's
reduced-precision A cannot be trusted to order near-ties at the k-th
boundary), and finishes the tiny CE / softmax epilogue.
"""

import numpy as np
import ml_dtypes

import concourse.bass as bass
import concourse.mybir as mybir
import concourse.tile as tile
from concourse import bacc
from concourse.bass_utils import run_bass_kernel_spmd

N_CORES = 8
N_TOTAL = 50000
ROWS = N_TOTAL // N_CORES          # 6250 per core
NB = 13                            # i-blocks per core
BLK = 512
ROWS_PAD = NB * BLK                # 6656
L = 1024                           # input feature dim
D1 = 512                           # feature dim
D2 = 256                           # attention dim
K_SAMPLE = 8
F32 = mybir.dt.float32
BF16 = mybir.dt.bfloat16
U32 = mybir.dt.uint32
PAD_START_IN_LAST = ROWS - (NB - 1) * BLK   # = 106

# Matmul dtype; float32r runs the PE at full rate (vs 1/4 for float32) at
# ~tf32 precision.  The BIR verifier requires every tensor consumed by an
# fp32r matmul to be produced as float32r, so matmul-feeding tiles are
# declared MM_DT (DMA and the scalar engine may produce it; the vector
# engine may not, hence the A-scores matmul runs on bf16 operands).
MM_DT = mybir.dt.float32r

LAST_RESULT = None  # BassKernelResults of the most recent kernel() call


def build_nc(bc_val: float):
    nc = bacc.Bacc("TRN2", target_bir_lowering=False, debug=False,
                   num_devices=N_CORES)

    hT = nc.dram_tensor("hT", [L, ROWS_PAD], BF16, kind="ExternalInput").ap()
    W1 = nc.dram_tensor("W1", [128, 8 * D1], BF16, kind="ExternalInput").ap()
    Wa = nc.dram_tensor("Wa", [128, 4 * D2], BF16, kind="ExternalInput").ap()
    Wb = nc.dram_tensor("Wb", [128, 4 * D2], BF16, kind="ExternalInput").ap()
    Wc = nc.dram_tensor("Wc", [128, 2], BF16, kind="ExternalInput").ap()
    ones = nc.dram_tensor("ones", [1, 128], MM_DT, kind="ExternalInput").ap()
    b1 = nc.dram_tensor("b1", [128, 4], F32, kind="ExternalInput").ap()
    ba = nc.dram_tensor("ba", [128, 2], F32, kind="ExternalInput").ap()
    bb = nc.dram_tensor("bb", [128, 2], F32, kind="ExternalInput").ap()

    A_out = nc.dram_tensor("A_out", [1, ROWS_PAD], F32, kind="ExternalOutput").ap()
    cvt_out = nc.dram_tensor("cvt_out", [1, NB * 8], F32, kind="ExternalOutput").ap()
    cit_out = nc.dram_tensor("cit_out", [1, NB * 8], U32, kind="ExternalOutput").ap()
    cvb_out = nc.dram_tensor("cvb_out", [1, NB * 8], F32, kind="ExternalOutput").ap()
    cib_out = nc.dram_tensor("cib_out", [1, NB * 8], U32, kind="ExternalOutput").ap()
    z_out = nc.dram_tensor("z_out", [128, 4], F32, kind="ExternalOutput").ap()
    ssum_out = nc.dram_tensor("ssum_out", [1, 1], F32, kind="ExternalOutput").ap()

    with tile.TileContext(nc) as tc:
        with (
            tc.tile_pool(name="const", bufs=1) as cpool,
            tc.tile_pool(name="persist", bufs=1) as ppool,
            tc.tile_pool(name="xt", bufs=4) as xtpool,
            tc.tile_pool(name="work", bufs=2) as wpool,
            tc.tile_pool(name="feat_ps", bufs=3, space="PSUM") as fpspool,
            tc.tile_pool(name="ag_ps", bufs=3, space="PSUM") as agpspool,
            tc.tile_pool(name="small_ps", bufs=1, space="PSUM") as spspool,
        ):
            # ---- resident constants -------------------------------------
            W1_sb = cpool.tile([128, 8, D1], BF16)       # [k_in_j, j, d1]
            nc.scalar.dma_start(W1_sb[:], W1.rearrange("p (j d) -> p j d", j=8))
            Wa_sb = cpool.tile([128, 4, D2], BF16)       # [d1_in_m, m, d2]
            nc.scalar.dma_start(Wa_sb[:], Wa.rearrange("p (m s) -> p m s", m=4))
            Wb_sb = cpool.tile([128, 4, D2], BF16)
            nc.scalar.dma_start(Wb_sb[:], Wb.rearrange("p (m s) -> p m s", m=4))
            Wc_sb = cpool.tile([128, 2], BF16)            # [d2_in_s, s]
            nc.scalar.dma_start(Wc_sb[:], Wc)
            b1_sb = cpool.tile([128, 4], F32)
            nc.scalar.dma_start(b1_sb[:], b1)
            ba_sb = cpool.tile([128, 2], F32)
            nc.scalar.dma_start(ba_sb[:], ba)
            bb_sb = cpool.tile([128, 2], F32)
            nc.scalar.dma_start(bb_sb[:], bb)
            ones_row = cpool.tile([1, 128], MM_DT)
            nc.scalar.dma_start(ones_row[:], ones)

            # ---- persistent accumulators --------------------------------
            A_all = ppool.tile([1, ROWS_PAD], F32)
            ssum_parts = ppool.tile([1, NB], F32)
            z_parts = ppool.tile([128, 4, NB], F32)
            cvt = ppool.tile([1, NB * 8], F32)
            cit = ppool.tile([1, NB * 8], U32)
            cvb = ppool.tile([1, NB * 8], F32)
            cib = ppool.tile([1, NB * 8], U32)

            hT_r = hT.rearrange("(j p) n -> p j n", p=128)

            for b in range(NB):
                last = b == NB - 1
                i0 = b * BLK
                isl = slice(i0, i0 + BLK)
                ksl = slice(b * 8, (b + 1) * 8)

                xt = xtpool.tile([128, 8, BLK], BF16, tag="xt")
                nc.sync.dma_start(xt[:], hT_r[:, :, isl])

                # featT[m] = relu(sum_j W1[j,m].T @ Xt[j] + b1[m])
                featT = wpool.tile([128, 4, BLK], BF16, tag="featT")
                for m in range(4):
                    fps = fpspool.tile([128, BLK], F32, tag="fps")
                    for j in range(8):
                        nc.tensor.matmul(
                            fps[:], W1_sb[:, j, m * 128:(m + 1) * 128],
                            xt[:, j, :], start=(j == 0), stop=(j == 7))
                    nc.scalar.activation(
                        featT[:, m, :], fps[:],
                        mybir.ActivationFunctionType.Relu,
                        bias=b1_sb[:, m:m + 1])

                # aT[s] = tanh(sum_m Wa[m,s].T @ featT[m] + ba[s]); gT likewise
                aT = wpool.tile([128, 2, BLK], F32, tag="aT")
                gT = wpool.tile([128, 2, BLK], F32, tag="gT")
                for dst, Wsb, bsb, fn in (
                    (aT, Wa_sb, ba_sb, mybir.ActivationFunctionType.Tanh),
                    (gT, Wb_sb, bb_sb, mybir.ActivationFunctionType.Sigmoid),
                ):
                    for s in range(2):
                        ps = agpspool.tile([128, BLK], F32, tag="agps")
                        for m in range(4):
                            nc.tensor.matmul(
                                ps[:], Wsb[:, m, s * 128:(s + 1) * 128],
                                featT[:, m, :], start=(m == 0), stop=(m == 3))
                        nc.scalar.activation(dst[:, s, :], ps[:], fn,
                                             bias=bsb[:, s:s + 1])

                agT = wpool.tile([128, 2, BLK], BF16, tag="agT")
                nc.vector.tensor_mul(agT[:], aT[:], gT[:])

                # A row = sum_s Wc[s].T @ agT[s]  (+ bc via activation bias)
                aps = spspool.tile([1, BLK], F32, tag="aps")
                for s in range(2):
                    nc.tensor.matmul(aps[:], Wc_sb[:, s:s + 1],
                                     agT[:, s, :], start=(s == 0), stop=(s == 1))
                nc.scalar.activation(A_all[0:1, isl], aps[:],
                                     mybir.ActivationFunctionType.Identity,
                                     bias=float(bc_val))

                # per-block top/bottom-8 candidates (values + in-block index);
                # the host adds the block offset and filters pad indices
                An = wpool.tile([1, BLK], F32, tag="An")
                nc.vector.tensor_scalar_mul(An[:], A_all[0:1, isl], -1.0)
                nc.vector.max(cvt[0:1, ksl], A_all[0:1, isl])
                nc.vector.max_index(cit[0:1, ksl], cvt[0:1, ksl], A_all[0:1, isl])
                nc.vector.max(cvb[0:1, ksl], An[:])
                nc.vector.max_index(cib[0:1, ksl], cvb[0:1, ksl], An[:])

                # sig row (+ partial sum); padded tail masked to 0
                sig = wpool.tile([1, BLK], MM_DT, tag="sig")
                if not last:
                    nc.scalar.activation(sig[:], aps[:],
                                         mybir.ActivationFunctionType.Sigmoid,
                                         bias=float(bc_val),
                                         accum_out=ssum_parts[0:1, b:b + 1])
                else:
                    nc.scalar.activation(sig[:], aps[:],
                                         mybir.ActivationFunctionType.Sigmoid,
                                         bias=float(bc_val))
                    nc.vector.memset(sig[0:1, PAD_START_IN_LAST:BLK].bitcast(F32), 0.0)
                    nc.vector.reduce_sum(ssum_parts[0:1, b:b + 1], sig[:].bitcast(F32),
                                         axis=mybir.AxisListType.X)

                # z partials: broadcast sig across partitions via PE, then
                # per-d1-slice multiply-reduce on DVE
                bps = spspool.tile([128, BLK], F32, tag="bps")
                nc.tensor.matmul(bps[:], ones_row[:], sig[:],
                                 start=True, stop=True)
                zscr = wpool.tile([128, BLK], F32, tag="zscr")
                for m in range(4):
                    nc.vector.scalar_tensor_tensor(
                        out=zscr[:], in0=featT[:, m, :], scalar=1.0,
                        in1=bps[:],
                        op0=mybir.AluOpType.mult, op1=mybir.AluOpType.mult,
                        accum_out=z_parts[:, m, b:b + 1])

            # ---- tails ---------------------------------------------------
            nc.gpsimd.dma_start(A_out, A_all[:])
            nc.gpsimd.dma_start(cvt_out, cvt[:])
            nc.gpsimd.dma_start(cit_out, cit[:])
            nc.gpsimd.dma_start(cvb_out, cvb[:])
            nc.gpsimd.dma_start(cib_out, cib[:])

            z_sb = ppool.tile([128, 4], F32)
            for m in range(4):
                nc.vector.reduce_sum(z_sb[:, m:m + 1], z_parts[:, m, :],
                                     axis=mybir.AxisListType.X)
            nc.gpsimd.dma_start(z_out, z_sb[:])

            ssum_sb = ppool.tile([1, 1], F32)
            nc.vector.reduce_sum(ssum_sb[:], ssum_parts[:],
                                 axis=mybir.AxisListType.X)
            nc.gpsimd.dma_start(ssum_out, ssum_sb[:])

    nc.compile()
    return nc


def kernel(h, W1, b1, Wa, ba, Wb, bb, Wc, bc, Wcls, bcls, Winst, binst, label):
    h = np.asarray(h, np.float32)
    W1 = np.asarray(W1, np.float32)
    Wa = np.asarray(Wa, np.float32)
    Wb = np.asarray(Wb, np.float32)
    Wc = np.asarray(Wc, np.float32)
    Wcls = np.asarray(Wcls, np.float32)
    Winst = np.asarray(Winst, np.float32)
    b1v = np.asarray(b1, np.float32).reshape(D1)
    bav = np.asarray(ba, np.float32).reshape(D2)
    bbv = np.asarray(bb, np.float32).reshape(D2)
    bc_val = float(np.asarray(bc, np.float32).reshape(-1)[0])

    nc = build_nc(bc_val)

    def klayout(w, parts):
        # [(t p), d] -> [p, t*d]: partition-major layout so the device DMA
        # is a flat contiguous [128, X] copy
        t = w.shape[0] // 128
        return np.ascontiguousarray(
            w.reshape(t, 128, -1).transpose(1, 0, 2).reshape(128, -1))

    shared = {
        "W1": klayout(W1, 8).astype(ml_dtypes.bfloat16),
        "Wa": klayout(Wa, 4).astype(ml_dtypes.bfloat16),
        "Wb": klayout(Wb, 4).astype(ml_dtypes.bfloat16),
        "Wc": klayout(Wc.reshape(D2, 1), 2).astype(ml_dtypes.bfloat16),
        "b1": klayout(b1v.reshape(D1, 1), 4).astype(np.float32),
        "ba": klayout(bav.reshape(D2, 1), 2).astype(np.float32),
        "bb": klayout(bbv.reshape(D2, 1), 2).astype(np.float32),
        "ones": np.ones((1, 128), np.float32),
    }
    in_maps = []
    for c in range(N_CORES):
        hT_c = np.zeros((L, ROWS_PAD), ml_dtypes.bfloat16)
        hT_c[:, :ROWS] = h[c * ROWS:(c + 1) * ROWS].T
        in_maps.append({"hT": np.ascontiguousarray(hT_c), **shared})

    res = run_bass_kernel_spmd(nc, in_maps, core_ids=list(range(N_CORES)))
    global LAST_RESULT
    LAST_RESULT = res

    # ---- host epilogue (gather / merge / tiny CE+softmax) ---------------
    z_full = np.zeros(D1, np.float64)
    ssum = 0.0
    cand = set()   # global indices nominated for the top side
    bcand = set()  # ... for the bottom side
    for c, r in enumerate(res.results):
        zc = r["z_out"]                      # [128, 4] -> z[m*128+p]
        z_full += zc.T.reshape(D1).astype(np.float64)
        ssum += float(r["ssum_out"][0, 0])
        ci_t = r["cit_out"][0].astype(np.int64)
        ci_b = r["cib_out"][0].astype(np.int64)
        for k in range(NB * 8):
            li = int(ci_t[k]) + (k // 8) * BLK
            if li < ROWS:
                cand.add(c * ROWS + li)
            li = int(ci_b[k]) + (k // 8) * BLK
            if li < ROWS:
                bcand.add(c * ROWS + li)

    # The device's reduced-precision A can flip near-tied candidates at the
    # k-th boundary, which visibly moves inst_loss.  Widen each side's
    # candidate set to the global top/bottom-64 by device A (the 8th<->64th
    # gap dwarfs the device A error), then re-rank the candidates with an
    # exact host recompute of their A scores and feature rows and run the
    # tiny CE on the exact values.
    A_dev = np.concatenate([r["A_out"][0, :ROWS] for r in res.results])
    WIDE = 64
    cand |= {int(g) for g in np.argpartition(-A_dev, WIDE)[:WIDE]}
    bcand |= {int(g) for g in np.argpartition(A_dev, WIDE)[:WIDE]}
    gidx = sorted(cand | bcand)
    hrows = h[gidx].astype(np.float64)
    feat_c = np.maximum(hrows @ W1 + b1v, 0)
    a_c = np.tanh(feat_c @ Wa + bav)
    g_c = 1.0 / (1.0 + np.exp(-(feat_c @ Wb + bbv)))
    A_c = ((a_c * g_c) @ Wc.reshape(D2, 1))[:, 0] + bc_val

    top_order = sorted((i for i, g in enumerate(gidx) if g in cand),
                       key=lambda i: (-A_c[i], gidx[i]))
    bot_order = sorted((i for i, g in enumerate(gidx) if g in bcand),
                       key=lambda i: (A_c[i], gidx[i]))
    sel = top_order[:K_SAMPLE] + bot_order[:K_SAMPLE]

    logits16 = feat_c[sel] @ Winst + np.asarray(binst, np.float64).reshape(1, 2)
    m = logits16.max(axis=1, keepdims=True)
    logp = logits16 - (m + np.log(np.exp(logits16 - m).sum(axis=1, keepdims=True)))
    targets = np.array([1] * K_SAMPLE + [0] * K_SAMPLE)
    inst_loss = -float(np.mean(logp[np.arange(2 * K_SAMPLE), targets]))
    inst_loss *= float(int(np.asarray(label).reshape(-1)[0]) == 1)

    zn = (z_full / ssum).astype(np.float32)          # [512] pooled feature
    slide_logits = (zn[None, :] @ Wcls + np.asarray(bcls, np.float32).reshape(1, 2))
    sl64 = slide_logits.astype(np.float64)
    e = np.exp(sl64 - sl64.max())
    slide_prob = (e / e.sum()).astype(np.float32)

    return (slide_logits.astype(np.float32),
            slide_prob.reshape(1, 2),
            np.float32(inst_loss))
